# Initial kernel scaffold
#
"""Trainium2 Bass kernel for BarycentricCoordinates (retrieval_knn) — v2.

Per (v, r) problem: nearest-neighbor ordering of 8 projected points vs a
template vertex, barycentric weights for every (second, third) pair,
Delaunay empty-circumcircle filter, min-score pair selection.

v2 redesign vs baseline:
 - Delaunay dets depend on r ONLY through the closest index c(r) in 0..7:
   computed once per v-tile at (ij=64, c=8, k=8) instead of per-r
   (5x less work), reduced over k, and packed into an 8-bit-per-pair
   validity table okall(ij) = sum_c validc * 2^c.  Per r the bit is
   extracted with okall * 2^-c(r) -> int -> &1.
 - valid pairs have w0+w1+w2 = 1 with all weights > 0 => score
   max(w^2) = (max w)^2, so argmin(max w) == argmin(max w^2): no squares.
 - single r-chunk (all 40 at once), reciprocal_approx_accurate (2 ULP),
   scalar_tensor_tensor / tensor_scalar fusions (tensor_scalar runs 2x on
   DVE even in fp32), one-hot argmin with value-equality one-hot.
 - w1(i,j) == w2(j,i) bitwise -> transposed views, single gather pair.
 - 8 cores data-parallel over V.
"""

import sys

sys.path.insert(0, "/opt/trn_rl_repo")

import numpy as np

import concourse.bass as bass
import concourse.bacc as bacc
import concourse.mybir as mybir
from concourse.tile import TileContext

F32 = mybir.dt.float32
I32 = mybir.dt.int32
I16 = mybir.dt.int16
OP = mybir.AluOpType
AF = mybir.ActivationFunctionType
AX = mybir.AxisListType

BIG = 2.0e38
SB = 1.0e6          # small-big offset for index packing
N_CORES = 8
V_TOTAL = 5000
R, A, K0 = 5, 8, 8
RA = R * A          # 40
VS = V_TOTAL // N_CORES
P = 128
VSP = 640
K2 = 64
RK = RA * K0        # 320
PP = RA * K2        # 2560
DET = K2 * K0 * K0  # 4096


def build_nc(vsp=VSP):
    nc = bacc.Bacc("TRN2", target_bir_lowering=False)
    n_vt = vsp // P

    px_d = nc.dram_tensor("px", (vsp, K0), F32, kind="ExternalInput")
    py_d = nc.dram_tensor("py", (vsp, K0), F32, kind="ExternalInput")
    tmpl_d = nc.dram_tensor("tmpl", (2, RA), F32, kind="ExternalInput")
    neqp_d = nc.dram_tensor("neqp", (1, 512), F32, kind="ExternalInput")
    iota8_d = nc.dram_tensor("iota8", (1, K0), F32, kind="ExternalInput")
    woff_d = nc.dram_tensor("woff", (1, RA), mybir.dt.uint32, kind="ExternalInput")
    outw_d = nc.dram_tensor("outw", (vsp, RA, 3), F32, kind="ExternalOutput")
    outi_d = nc.dram_tensor("outi", (vsp, RA, 3), F32, kind="ExternalOutput")

    with TileContext(nc) as tc:
        VE = nc.vector
        GP = nc.gpsimd
        SC = nc.scalar

        def bcv(ap, shape):
            return ap.to_broadcast(shape)

        with (
            tc.tile_pool(name="const", bufs=1) as cpool,
            tc.tile_pool(name="vt", bufs=2) as vpool,
            tc.tile_pool(name="det", bufs=2) as dpool,
            tc.tile_pool(name="ij", bufs=1) as ipool,
            tc.tile_pool(name="rk", bufs=1) as rkpool,
            tc.tile_pool(name="small", bufs=2) as opool,
        ):
            TX = cpool.tile([P, RA], F32, tag="TX")
            TY = cpool.tile([P, RA], F32, tag="TY")
            NEQP = cpool.tile([P, 512], F32, tag="NEQP")
            IOTA8 = cpool.tile([P, K0], F32, tag="IOTA8")
            WOFF = cpool.tile([P, RA], mybir.dt.uint32, tag="WOFF")
            nc.sync.dma_start(TX, tmpl_d[0:1, :].to_broadcast((P, RA)))
            nc.sync.dma_start(TY, tmpl_d[1:2, :].to_broadcast((P, RA)))
            nc.sync.dma_start(NEQP, neqp_d[0:1, :].to_broadcast((P, 512)))
            nc.sync.dma_start(IOTA8, iota8_d[0:1, :].to_broadcast((P, K0)))
            nc.sync.dma_start(WOFF, woff_d[0:1, :].to_broadcast((P, RA)))

            for vt in range(n_vt):
                v0_, v1_ = vt * P, (vt + 1) * P
                px = vpool.tile([P, K0], F32, tag="px")
                py = vpool.tile([P, K0], F32, tag="py")
                nc.sync.dma_start(px, px_d[v0_:v1_, :])
                nc.sync.dma_start(py, py_d[v0_:v1_, :])

                # ---- s = |p|^2 ----
                t8a = vpool.tile([P, K0], F32, tag="t8a")
                t8b = vpool.tile([P, K0], F32, tag="t8b")
                s_ = vpool.tile([P, K0], F32, tag="s")
                SC.activation(out=t8a, in_=px, func=AF.Square)
                SC.activation(out=t8b, in_=py, func=AF.Square)
                VE.tensor_tensor(out=s_, in0=t8a, in1=t8b, op=OP.add)

                # ---- b-tensors b(c,k) = p_c - p_k (also used as (i,k)/(j,k)) ----
                bx = vpool.tile([P, K2], F32, tag="bx")
                by = vpool.tile([P, K2], F32, tag="by")
                bs = vpool.tile([P, K2], F32, tag="bs")
                bxv = bx.rearrange("p (i k) -> p i k", k=K0)
                byv = by.rearrange("p (i k) -> p i k", k=K0)
                bsv = bs.rearrange("p (i k) -> p i k", k=K0)
                VE.tensor_tensor(out=bxv, in0=bcv(px.unsqueeze(2), (P, K0, K0)),
                                 in1=bcv(px.unsqueeze(1), (P, K0, K0)), op=OP.subtract)
                GP.tensor_tensor(out=byv, in0=bcv(py.unsqueeze(2), (P, K0, K0)),
                                 in1=bcv(py.unsqueeze(1), (P, K0, K0)), op=OP.subtract)
                VE.tensor_tensor(out=bsv, in0=bcv(s_.unsqueeze(2), (P, K0, K0)),
                                 in1=bcv(s_.unsqueeze(1), (P, K0, K0)), op=OP.subtract)

                # ---- U cross tensors (i,j,k) = (ij, k) ----
                def Bi(t):
                    return bcv(t.rearrange("p (i k) -> p i k", k=K0).unsqueeze(2),
                               (P, K0, K0, K0))

                def Bj(t):
                    return bcv(t.rearrange("p (j k) -> p j k", k=K0).unsqueeze(1),
                               (P, K0, K0, K0))

                U1 = vpool.tile([P, 512], F32, tag="U1")
                U2 = vpool.tile([P, 512], F32, tag="U2")
                U3 = vpool.tile([P, 512], F32, tag="U3")
                uA = vpool.tile([P, 512], F32, tag="uA")
                U1v = U1.rearrange("p (i j k) -> p i j k", j=K0, k=K0)
                U2v = U2.rearrange("p (i j k) -> p i j k", j=K0, k=K0)
                U3v = U3.rearrange("p (i j k) -> p i j k", j=K0, k=K0)
                uAv = uA.rearrange("p (i j k) -> p i j k", j=K0, k=K0)
                VE.tensor_tensor(out=U1v, in0=Bi(by), in1=Bj(bs), op=OP.mult)
                GP.tensor_tensor(out=uAv, in0=Bi(bs), in1=Bj(by), op=OP.mult)
                VE.tensor_tensor(out=U1, in0=U1, in1=uA, op=OP.subtract)
                GP.tensor_tensor(out=U2v, in0=Bi(bx), in1=Bj(bs), op=OP.mult)
                GP.tensor_tensor(out=uAv, in0=Bi(bs), in1=Bj(bx), op=OP.mult)
                VE.tensor_tensor(out=U2, in0=U2, in1=uA, op=OP.subtract)
                VE.tensor_tensor(out=U3v, in0=Bi(bx), in1=Bj(by), op=OP.mult)
                GP.tensor_tensor(out=uAv, in0=Bi(by), in1=Bj(bx), op=OP.mult)
                VE.tensor_tensor(out=U3, in0=U3, in1=uA, op=OP.subtract)

                # ---- dets E(ij, c, k) = bx*U1 - by*U2 + bs*U3; keep iff
                #      max_k E <= 0 (== baseline det' >= 0 with v0 = -b) ----
                def Uv(t):
                    return bcv(t.rearrange("p (q k) -> p q k", k=K0).unsqueeze(2),
                               (P, K2, K0, K0))

                def Bc(t):
                    return bcv(t.rearrange("p (c k) -> p c k", k=K0).unsqueeze(1),
                               (P, K2, K0, K0))

                # pack U into 48-pair blocks: A=(0:4,4:8) q0:16, B=(0:4,0:4)
                # q16:32, C=(4:8,4:8) q32:48; D=(4:8,0:4) comes from A by
                # antisymmetry: ok(j,i) <=> min_k E(i,j,k) >= 0
                Q48 = 48
                U48s = []
                for un, (Ut, Utv) in enumerate(((U1, U1v), (U2, U2v), (U3, U3v))):
                    t48 = vpool.tile([P, Q48 * K0], F32, tag=f"U48_{un}")
                    for b, (io, jo) in enumerate(((0, 4), (0, 0), (4, 4))):
                        SC.copy(out=t48[:, b * 128:(b + 1) * 128].rearrange(
                            "p (a b2 k) -> p a b2 k", b2=4, k=K0),
                                in_=Utv[:, io:io + 4, jo:jo + 4, :])
                    U48s.append(t48)
                maxE = vpool.tile([P, Q48 * K0], F32, tag="maxE")
                maxEv = maxE.rearrange("p (q c) -> p q c", c=K0)
                minEA = vpool.tile([P, 16 * K0], F32, tag="minEA")
                minEAv = minEA.rearrange("p (q c) -> p q c", c=K0)
                CH = K0 // 2
                for ch in range(2):
                    c0 = ch * CH
                    e1 = dpool.tile([P, Q48 * CH * K0], F32, tag="e1")
                    e2 = dpool.tile([P, Q48 * CH * K0], F32, tag="e2")
                    e1v = e1.rearrange("p (q c k) -> p q c k", c=CH, k=K0)
                    e2v = e2.rearrange("p (q c k) -> p q c k", c=CH, k=K0)

                    def Bch(t, c0=c0):
                        return bcv(t.rearrange("p (c k) -> p c k", k=K0)
                                   [:, c0:c0 + CH, :].unsqueeze(1),
                                   (P, Q48, CH, K0))

                    def Uch(t):
                        return bcv(t.rearrange("p (q k) -> p q k", k=K0)
                                   .unsqueeze(2), (P, Q48, CH, K0))

                    VE.tensor_tensor(out=e1v, in0=Uch(U48s[0]), in1=Bch(bx),
                                     op=OP.mult)
                    GP.tensor_tensor(out=e2v, in0=Uch(U48s[1]), in1=Bch(by),
                                     op=OP.mult)
                    GP.tensor_tensor(out=e1, in0=e1, in1=e2, op=OP.subtract)
                    VE.tensor_tensor(out=e2v, in0=Uch(U48s[2]), in1=Bch(bs),
                                     op=OP.mult)
                    GP.tensor_tensor(out=e1, in0=e1, in1=e2, op=OP.add)
                    VE.tensor_reduce(out=maxEv[:, :, c0:c0 + CH], in_=e1v,
                                     axis=AX.X, op=OP.max)
                    VE.tensor_reduce(out=minEAv[:, :, c0:c0 + CH],
                                     in_=e1v[:, 0:16, :, :], axis=AX.X,
                                     op=OP.min)
                # masks then scatter into (ij, c) validity table
                mskF = vpool.tile([P, Q48 * K0], F32, tag="mskF")
                mskD = vpool.tile([P, 16 * K0], F32, tag="mskD")
                VE.tensor_scalar(out=mskF, in0=maxE, scalar1=0.0, scalar2=None,
                                 op0=OP.is_le)
                VE.tensor_scalar(out=mskD, in0=minEA, scalar1=0.0, scalar2=None,
                                 op0=OP.is_ge)
                okb = vpool.tile([P, 512], F32, tag="okb")
                okbv = okb.rearrange("p (i j c) -> p i j c", j=K0, c=K0)
                mFv = mskF.rearrange("p (q c) -> p q c", c=K0)
                SC.copy(out=okbv[:, 0:4, 4:8, :], in_=mFv[:, 0:16, :].rearrange(
                    "p (a b) c -> p a b c", b=4))
                SC.copy(out=okbv[:, 0:4, 0:4, :], in_=mFv[:, 16:32, :].rearrange(
                    "p (a b) c -> p a b c", b=4))
                SC.copy(out=okbv[:, 4:8, 4:8, :], in_=mFv[:, 32:48, :].rearrange(
                    "p (a b) c -> p a b c", b=4))
                SC.copy(out=okbv[:, 4:8, 0:4, :], in_=mskD.rearrange(
                    "p (a b c) -> p b a c", b=4, c=K0))
                GP.tensor_tensor(out=okb, in0=okb, in1=NEQP, op=OP.mult)
                okall = vpool.tile([P, K2], F32, tag="okall")
                VE.tensor_reduce(out=okall,
                                 in_=okb.rearrange("p (q c) -> p q c", c=K0),
                                 axis=AX.X, op=OP.add)

                # ---- per-r: distances / closest ----
                px_rk = bcv(px.unsqueeze(1), (P, RA, K0))
                py_rk = bcv(py.unsqueeze(1), (P, RA, K0))
                tx_rk = bcv(TX.unsqueeze(2), (P, RA, K0))
                ty_rk = bcv(TY.unsqueeze(2), (P, RA, K0))
                tdx = rkpool.tile([P, RK], F32, tag="tdx")
                tdy = rkpool.tile([P, RK], F32, tag="tdy")
                d2 = rkpool.tile([P, RK], F32, tag="d2")
                tdxv = tdx.rearrange("p (r k) -> p r k", k=K0)
                tdyv = tdy.rearrange("p (r k) -> p r k", k=K0)
                d2v = d2.rearrange("p (r k) -> p r k", k=K0)
                GP.tensor_tensor(out=tdxv, in0=px_rk, in1=tx_rk, op=OP.subtract)
                GP.tensor_tensor(out=tdyv, in0=py_rk, in1=ty_rk, op=OP.subtract)
                SC.activation(out=tdx, in_=tdx, func=AF.Square)
                SC.activation(out=tdy, in_=tdy, func=AF.Square)
                GP.tensor_tensor(out=d2, in0=tdx, in1=tdy, op=OP.add)
                dmin = opool.tile([P, RA], F32, tag="dmin")
                VE.tensor_reduce(out=dmin, in_=d2v, axis=AX.X, op=OP.min)
                dmin_rk = bcv(dmin.unsqueeze(2), (P, RA, K0))
                m0 = rkpool.tile([P, RK], F32, tag="m0")
                m0v = m0.rearrange("p (r k) -> p r k", k=K0)
                VE.tensor_tensor(out=m0v, in0=d2v, in1=dmin_rk, op=OP.is_equal)

                # gather streams: cx, cy, c_f (iota8)
                STRM = vpool.tile([P, 24], F32, tag="STRM")
                SC.copy(out=STRM[:, 0:8], in_=px)
                SC.copy(out=STRM[:, 8:16], in_=py)
                SC.copy(out=STRM[:, 16:24], in_=IOTA8)
                gm = rkpool.tile([P, RK * 3], F32, tag="gm")
                gmv = gm.rearrange("p (r g k) -> p r g k", g=3, k=K0)
                GP.tensor_tensor(
                    out=gmv, in0=bcv(m0v.unsqueeze(2), (P, RA, 3, K0)),
                    in1=bcv(STRM.rearrange("p (g k) -> p g k", k=K0).unsqueeze(1),
                            (P, RA, 3, K0)), op=OP.mult)
                g4 = opool.tile([P, RA * 3], F32, tag="g4")
                g4v = g4.rearrange("p (r g) -> p r g", g=3)
                VE.tensor_reduce(out=g4v, in_=gmv, axis=AX.X, op=OP.add)
                cx = g4v[:, :, 0:1].squeeze(2)
                cy = g4v[:, :, 1:2].squeeze(2)
                c_f = g4v[:, :, 2:3].squeeze(2)

                # second closest (for fallback index)
                d2b = rkpool.tile([P, RK], F32, tag="d2b")
                VE.scalar_tensor_tensor(out=d2b, in0=m0, scalar=BIG, in1=d2,
                                        op0=OP.mult, op1=OP.add)
                dmin2 = opool.tile([P, RA], F32, tag="dmin2")
                d2bv = d2b.rearrange("p (r k) -> p r k", k=K0)
                VE.tensor_reduce(out=dmin2, in_=d2bv, axis=AX.X, op=OP.min)
                eq2 = rkpool.tile([P, RK], F32, tag="eq2")
                eq2v = eq2.rearrange("p (r k) -> p r k", k=K0)
                VE.tensor_tensor(out=eq2v, in0=d2bv,
                                 in1=bcv(dmin2.unsqueeze(2), (P, RA, K0)),
                                 op=OP.is_equal)
                GP.tensor_tensor(out=eq2v, in0=eq2v,
                                 in1=bcv(IOTA8.unsqueeze(1), (P, RA, K0)),
                                 op=OP.mult)
                o1_f = opool.tile([P, RA], F32, tag="o1_f")
                VE.tensor_reduce(out=o1_f, in_=eq2v, axis=AX.X, op=OP.add)

                # ---- v0, d00, d02 ----
                v0x = rkpool.tile([P, RK], F32, tag="v0x")
                v0y = rkpool.tile([P, RK], F32, tag="v0y")
                v0xv = v0x.rearrange("p (r k) -> p r k", k=K0)
                v0yv = v0y.rearrange("p (r k) -> p r k", k=K0)
                GP.tensor_tensor(out=v0xv, in0=px_rk,
                                 in1=bcv(cx.unsqueeze(2), (P, RA, K0)),
                                 op=OP.subtract)
                GP.tensor_tensor(out=v0yv, in0=py_rk,
                                 in1=bcv(cy.unsqueeze(2), (P, RA, K0)),
                                 op=OP.subtract)
                q1 = rkpool.tile([P, RK], F32, tag="q1")
                q2 = rkpool.tile([P, RK], F32, tag="q2")
                a2 = rkpool.tile([P, RK], F32, tag="a2")
                v2x = opool.tile([P, RA], F32, tag="v2x")
                v2y = opool.tile([P, RA], F32, tag="v2y")
                VE.tensor_tensor(out=v2x, in0=TX, in1=cx, op=OP.subtract)
                VE.tensor_tensor(out=v2y, in0=TY, in1=cy, op=OP.subtract)
                # A2'(r,k) = cross(v2, v0_k) = v2x*v0y_k - v2y*v0x_k
                GP.tensor_tensor(out=q1.rearrange("p (r k) -> p r k", k=K0),
                                 in0=v0yv, in1=bcv(v2x.unsqueeze(2), (P, RA, K0)),
                                 op=OP.mult)
                GP.tensor_tensor(out=q2.rearrange("p (r k) -> p r k", k=K0),
                                 in0=v0xv, in1=bcv(v2y.unsqueeze(2), (P, RA, K0)),
                                 op=OP.mult)
                VE.tensor_tensor(out=a2, in0=q1, in1=q2, op=OP.subtract)

                # ---- (r, i, j) chain ----
                def XI(t2):
                    return bcv(t2.rearrange("p (r k) -> p r k", k=K0).unsqueeze(3),
                               (P, RA, K0, K0))

                def XJ(t2):
                    return bcv(t2.rearrange("p (r k) -> p r k", k=K0).unsqueeze(2),
                               (P, RA, K0, K0))

                def T4(t):
                    return t.rearrange("p (r i j) -> p r i j", i=K0, j=K0)

                def T4T(t):
                    return t.rearrange("p (r j i) -> p r i j", j=K0, i=K0)

                s0 = ipool.tile([P, PP], F32, tag="s0")
                s1 = ipool.tile([P, PP], F32, tag="s1")
                s2 = ipool.tile([P, PP], F32, tag="s2")
                s3 = ipool.tile([P, PP], F32, tag="s3")
                s4 = ipool.tile([P, PP], F32, tag="s4")
                s5 = ipool.tile([P, PP], F32, tag="s5")
                s6 = ipool.tile([P, PP], F32, tag="s6")
                w2 = ipool.tile([P, PP], F32, tag="w2")
                si = ipool.tile([P, PP], I16, tag="si")

                # CR(r,i,j) = cross(v0_i, v0_j) on 48-pair blocks only;
                # D=(4:8,0:4) inverse = exact negated transpose of A's.
                # w2 = A2'(j) / CR(i,j); no clamp needed: w2 feeds only
                # NaN-tolerant min/max/compare chains (outputs come from
                # the exact refine below).
                QW = RA * Q48
                v0xv2 = v0x.rearrange("p (r k) -> p r k", k=K0)
                v0yv2 = v0y.rearrange("p (r k) -> p r k", k=K0)
                a2v2 = a2.rearrange("p (r k) -> p r k", k=K0)

                def blkI(t, io):
                    return bcv(t[:, :, io:io + 4].unsqueeze(3), (P, RA, 4, 4))

                def blkJ(t, jo):
                    return bcv(t[:, :, jo:jo + 4].unsqueeze(2), (P, RA, 4, 4))

                def q48blk(t, b):
                    return t[:, 0:QW].rearrange("p (r q) -> p r q", q=Q48)[
                        :, :, b * 16:(b + 1) * 16].rearrange(
                        "p r (a b2) -> p r a b2", b2=4)

                BLKS = ((0, 4), (0, 0), (4, 4))
                for b, (io, jo) in enumerate(BLKS):
                    GP.tensor_tensor(out=q48blk(s0, b), in0=blkI(v0xv2, io),
                                     in1=blkJ(v0yv2, jo), op=OP.mult)
                for b, (io, jo) in enumerate(BLKS):
                    GP.tensor_tensor(out=q48blk(s1, b), in0=blkI(v0yv2, io),
                                     in1=blkJ(v0xv2, jo), op=OP.mult)
                GP.tensor_tensor(out=s2[:, 0:QW], in0=s0[:, 0:QW],
                                 in1=s1[:, 0:QW], op=OP.subtract)   # CR48
                VE.reciprocal_approx_accurate(out=s3[:, 0:QW], in_=s2[:, 0:QW],
                                              scratch=s4[:, 0:QW])
                w2v4 = w2.rearrange("p (r i j) -> p r i j", i=K0, j=K0)
                for b, (io, jo) in enumerate(BLKS):
                    GP.tensor_tensor(out=w2v4[:, :, io:io + 4, jo:jo + 4],
                                     in0=blkJ(a2v2, jo), in1=q48blk(s3, b),
                                     op=OP.mult)
                # D block: inv(i,j) = -invA(j, i-4); w2D = a2_j * invA^T, negated
                invAT = s3[:, 0:QW].rearrange("p (r q) -> p r q", q=Q48)[
                    :, :, 0:16].rearrange("p r (a b2) -> p r b2 a", b2=4)
                GP.tensor_tensor(out=w2v4[:, :, 4:8, 0:4], in0=blkJ(a2v2, 0),
                                 in1=invAT, op=OP.mult)
                VE.tensor_scalar(out=w2v4[:, :, 4:8, 0:4],
                                 in0=w2v4[:, :, 4:8, 0:4], scalar1=-1.0,
                                 scalar2=None, op0=OP.mult)
                # w1 = transpose(w2); w0 = 1 - w2 - w1
                GP.tensor_tensor(out=T4(s1), in0=T4(w2), in1=T4T(w2), op=OP.add)
                SC.activation(out=s2, in_=s1, func=AF.Copy, bias=1.0, scale=-1.0)
                VE.tensor_tensor(out=T4(s3), in0=T4(w2), in1=T4T(w2), op=OP.min)
                VE.tensor_tensor(out=T4(s6), in0=T4(w2), in1=T4T(w2), op=OP.max)
                VE.tensor_tensor(out=s5, in0=s6, in1=s2, op=OP.max)  # wmax
                VE.tensor_tensor(out=s6, in0=s3, in1=s2, op=OP.min)  # wmin

                # ---- okbit extract: (okall_int >> c) & 1 -> float ----
                # all casts are integral-valued (no trunc-vs-round ambiguity)
                oki = vpool.tile([P, K2], I16, tag="oki")
                VE.tensor_copy(out=oki, in_=okall)
                ci = opool.tile([P, RA], I16, tag="ci")
                VE.tensor_copy(out=ci, in_=c_f)
                VE.tensor_tensor(out=si.rearrange("p (r q) -> p r q", q=K2),
                                 in0=bcv(oki.unsqueeze(1), (P, RA, K2)),
                                 in1=bcv(ci.unsqueeze(2), (P, RA, K2)),
                                 op=OP.logical_shift_right)
                VE.tensor_scalar(out=si, in0=si, scalar1=1, scalar2=None,
                                 op0=OP.bitwise_and)
                SC.copy(out=s1, in_=si)                               # bitf
                # valid = (wmin > 0) * bitf ; score = max(wmax, (1-valid)*BIG)
                VE.scalar_tensor_tensor(out=s2, in0=s6, scalar=0.0, in1=s1,
                                        op0=OP.is_gt, op1=OP.mult)
                SC.activation(out=s1, in_=s2, func=AF.Copy, bias=BIG, scale=-BIG)
                VE.tensor_tensor(out=s2, in0=s5, in1=s1, op=OP.max)   # score
                smin = opool.tile([P, RA], F32, tag="smin")
                VE.tensor_reduce(out=smin,
                                 in_=s2.rearrange("p (r q) -> p r q", q=K2),
                                 axis=AX.X, op=OP.min)
                # windowed argmin: 5 windows of 8 r-rows (512 lanes each);
                # max_index returns the first lane matching each smin value
                idxw = opool.tile([P, RA], mybir.dt.uint32, tag="idxw")
                for qw in range(RA // 8):
                    VE.max_index(out=idxw[:, 8 * qw:8 * qw + 8],
                                 in_max=smin[:, 8 * qw:8 * qw + 8],
                                 in_values=s2[:, 512 * qw:512 * (qw + 1)])
                ij_i = opool.tile([P, RA], I32, tag="ij_i")
                VE.tensor_tensor(out=ij_i, in0=idxw, in1=WOFF, op=OP.subtract)

                # ---- tail: fallback, index decode, outputs ----
                fb = opool.tile([P, RA], F32, tag="fb")
                nfb = opool.tile([P, RA], F32, tag="nfb")
                VE.tensor_scalar(out=fb, in0=smin, scalar1=1.0e38, scalar2=None,
                                 op0=OP.is_ge)
                VE.tensor_scalar(out=nfb, in0=fb, scalar1=-1.0, scalar2=1.0,
                                 op0=OP.mult, op1=OP.add)
                i_i = opool.tile([P, RA], I32, tag="i_i")
                j_i = opool.tile([P, RA], I32, tag="j_i")
                i_f = opool.tile([P, RA], F32, tag="i_f")
                j_f = opool.tile([P, RA], F32, tag="j_f")
                VE.tensor_scalar(out=i_i, in0=ij_i, scalar1=3, scalar2=None,
                                 op0=OP.arith_shift_right)
                VE.tensor_scalar(out=j_i, in0=ij_i, scalar1=7, scalar2=None,
                                 op0=OP.bitwise_and)
                VE.tensor_copy(out=i_f, in_=i_i)
                VE.tensor_copy(out=j_f, in_=j_i)

                # recompute selected-pair weights at (r)-width in the
                # reference's exact dot-product operation order (gathering
                # v0 at i*, j* with 8-wide one-hots)
                i8 = bcv(IOTA8.unsqueeze(1), (P, RA, K0))
                ohi = rkpool.tile([P, RK], F32, tag="ohi")
                ohj = rkpool.tile([P, RK], F32, tag="ohj")
                ohiv = ohi.rearrange("p (r k) -> p r k", k=K0)
                ohjv = ohj.rearrange("p (r k) -> p r k", k=K0)
                VE.tensor_tensor(out=ohiv, in0=i8,
                                 in1=bcv(i_f.unsqueeze(2), (P, RA, K0)),
                                 op=OP.is_equal)
                VE.tensor_tensor(out=ohjv, in0=i8,
                                 in1=bcv(j_f.unsqueeze(2), (P, RA, K0)),
                                 op=OP.is_equal)
                gx = opool.tile([P, 4 * RA], F32, tag="gx")
                gxv = gx.rearrange("p (g r) -> p g r", g=4)
                grk = rkpool.tile([P, 4 * RK], F32, tag="grk")
                grkv = grk.rearrange("p (g r k) -> p g r k", g=4, k=K0)
                GP.tensor_tensor(out=grkv[:, 0, :, :], in0=ohiv, in1=v0xv,
                                 op=OP.mult)
                GP.tensor_tensor(out=grkv[:, 1, :, :], in0=ohiv, in1=v0yv,
                                 op=OP.mult)
                GP.tensor_tensor(out=grkv[:, 2, :, :], in0=ohjv, in1=v0xv,
                                 op=OP.mult)
                GP.tensor_tensor(out=grkv[:, 3, :, :], in0=ohjv, in1=v0yv,
                                 op=OP.mult)
                VE.tensor_reduce(out=gxv, in_=grk.rearrange(
                    "p (gr k) -> p gr k", k=K0), axis=AX.X, op=OP.add)
                xi = gxv[:, 0, :]
                yi = gxv[:, 1, :]
                xj = gxv[:, 2, :]
                yj = gxv[:, 3, :]
                ta = opool.tile([P, RA], F32, tag="ta")
                tb = opool.tile([P, RA], F32, tag="tb")
                sqg = opool.tile([P, 4 * RA], F32, tag="sqg")
                d00ij = opool.tile([P, 2 * RA], F32, tag="d00ij")
                SC.activation(out=sqg, in_=gx, func=AF.Square)
                sq4 = sqg.rearrange("p (a b r) -> p a b r", a=2, b=2)
                VE.tensor_tensor(out=d00ij.rearrange("p (a r) -> p a r", a=2),
                                 in0=sq4[:, :, 0, :], in1=sq4[:, :, 1, :],
                                 op=OP.add)
                d00i = d00ij[:, 0:RA]
                d00j = d00ij[:, RA:2 * RA]
                P6 = opool.tile([P, 6 * RA], F32, tag="P6")
                P6v = P6.rearrange("p (g r) -> p g r", g=6)
                VE.tensor_tensor(out=P6v[:, 0, :], in0=xi, in1=xj, op=OP.mult)
                VE.tensor_tensor(out=P6v[:, 1, :], in0=xi, in1=v2x, op=OP.mult)
                VE.tensor_tensor(out=P6v[:, 2, :], in0=xj, in1=v2x, op=OP.mult)
                GP.tensor_tensor(out=P6v[:, 3, :], in0=yi, in1=yj, op=OP.mult)
                GP.tensor_tensor(out=P6v[:, 4, :], in0=yi, in1=v2y, op=OP.mult)
                GP.tensor_tensor(out=P6v[:, 5, :], in0=yj, in1=v2y, op=OP.mult)
                D3 = opool.tile([P, 3 * RA], F32, tag="D3")
                P6h = P6.rearrange("p (h g r) -> p h g r", h=2, g=3)
                VE.tensor_tensor(out=D3.rearrange("p (g r) -> p g r", g=3),
                                 in0=P6h[:, 0, :, :], in1=P6h[:, 1, :, :],
                                 op=OP.add)
                dot01r = D3[:, 0:RA]
                d02i = D3[:, RA:2 * RA]
                d02j = D3[:, 2 * RA:3 * RA]
                denr = opool.tile([P, RA], F32, tag="denr")
                invr = opool.tile([P, RA], F32, tag="invr")
                scr = opool.tile([P, RA], F32, tag="scr")
                SC.activation(out=ta, in_=dot01r, func=AF.Square)
                VE.tensor_tensor(out=tb, in0=d00i, in1=d00j, op=OP.mult)
                VE.tensor_tensor(out=denr, in0=tb, in1=ta, op=OP.subtract)
                VE.scalar_tensor_tensor(out=denr, in0=denr, scalar=0.0,
                                        in1=denr, op0=OP.is_equal, op1=OP.add)
                VE.reciprocal_approx_accurate(out=invr, in_=denr, scratch=scr)
                P4 = opool.tile([P, 4 * RA], F32, tag="P4")
                P4v = P4.rearrange("p (g r) -> p g r", g=4)
                VE.tensor_tensor(out=P4v[:, 0, :], in0=d00j, in1=d02i, op=OP.mult)
                VE.tensor_tensor(out=P4v[:, 1, :], in0=d00i, in1=d02j, op=OP.mult)
                GP.tensor_tensor(out=P4v[:, 2, :], in0=dot01r, in1=d02j, op=OP.mult)
                GP.tensor_tensor(out=P4v[:, 3, :], in0=dot01r, in1=d02i, op=OP.mult)
                wsel = opool.tile([P, 2 * RA], F32, tag="wsel")
                wselv = wsel.rearrange("p (g r) -> p g r", g=2)
                P4h = P4.rearrange("p (h g r) -> p h g r", h=2, g=2)
                VE.tensor_tensor(out=wselv, in0=P4h[:, 0, :, :],
                                 in1=P4h[:, 1, :, :], op=OP.subtract)
                VE.tensor_tensor(out=wselv, in0=wselv,
                                 in1=bcv(invr.unsqueeze(1), (P, 2, RA)),
                                 op=OP.mult)
                w2sel = wsel[:, 0:RA]
                w1sel = wsel[:, RA:2 * RA]

                VE.copy_predicated(out=i_f, mask=fb.bitcast(I32), data=o1_f)
                VE.copy_predicated(out=j_f, mask=fb.bitcast(I32), data=o1_f)

                w2o = opool.tile([P, RA], F32, tag="w2o")
                w1o = opool.tile([P, RA], F32, tag="w1o")
                w0o = opool.tile([P, RA], F32, tag="w0o")
                VE.tensor_tensor(out=w2o, in0=w2sel, in1=nfb, op=OP.mult)
                VE.tensor_tensor(out=w1o, in0=w1sel, in1=nfb, op=OP.mult)
                VE.tensor_tensor(out=w0o, in0=w2o, in1=w1o, op=OP.add)
                VE.tensor_scalar(out=w0o, in0=w0o, scalar1=-1.0, scalar2=1.0,
                                 op0=OP.mult, op1=OP.add)
                VE.tensor_tensor(out=w0o, in0=w0o, in1=nfb, op=OP.mult)


                wout = opool.tile([P, RA * 3], F32, tag="wout")
                iout = opool.tile([P, RA * 3], F32, tag="iout")
                woutv = wout.rearrange("p (r c) -> p r c", c=3)
                ioutv = iout.rearrange("p (r c) -> p r c", c=3)
                SC.copy(out=woutv[:, :, 0], in_=w0o)
                SC.copy(out=woutv[:, :, 1], in_=w2o)
                SC.copy(out=woutv[:, :, 2], in_=w1o)
                SC.copy(out=ioutv[:, :, 0], in_=c_f)
                SC.copy(out=ioutv[:, :, 1], in_=i_f)
                SC.copy(out=ioutv[:, :, 2], in_=j_f)
                nc.sync.dma_start(outw_d[v0_:v1_, :, :], woutv)
                nc.sync.dma_start(outi_d[v0_:v1_, :, :], ioutv)

    nc.compile()
    return nc


def make_consts():
    i = np.arange(K0)
    ii = (np.arange(K2) // K0)[:, None]
    jj = (np.arange(K2) % K0)[:, None]
    cc = i[None, :]
    neq = (ii != jj) & (ii != cc) & (jj != cc)          # (64, 8)
    neqp = (neq * (2.0 ** cc)).astype(np.float32).reshape(1, 512)
    iota8 = i.astype(np.float32).reshape(1, K0)
    woff = ((np.arange(RA) % 8) * K2).astype(np.uint32).reshape(1, RA)
    return {"neqp": neqp, "iota8": iota8, "woff": woff}


def make_in_maps(template, projections):
    template = np.ascontiguousarray(np.asarray(template, np.float32))
    projections = np.ascontiguousarray(np.asarray(projections, np.float32))
    consts = make_consts()
    tmplT = np.stack([template[..., 0].reshape(-1), template[..., 1].reshape(-1)])
    px_all = np.ascontiguousarray(projections[..., 0])
    py_all = np.ascontiguousarray(projections[..., 1])
    in_maps = []
    for c in range(N_CORES):
        pxc = px_all[c * VS:(c + 1) * VS]
        pyc = py_all[c * VS:(c + 1) * VS]
        pad = VSP - VS
        pxc = np.concatenate([pxc, np.broadcast_to(pxc[:1], (pad, K0))], 0)
        pyc = np.concatenate([pyc, np.broadcast_to(pyc[:1], (pad, K0))], 0)
        m = {"px": np.ascontiguousarray(pxc), "py": np.ascontiguousarray(pyc),
             "tmpl": tmplT}
        m.update(consts)
        in_maps.append(m)
    return in_maps


_NC_CACHE = {}


def kernel(template, projections, _want_time=False):
    from concourse.bass_utils import run_bass_kernel_spmd
    if "nc" not in _NC_CACHE:
        _NC_CACHE["nc"] = build_nc()
    nc = _NC_CACHE["nc"]
    in_maps = make_in_maps(template, projections)
    res = run_bass_kernel_spmd(nc, in_maps, core_ids=list(range(N_CORES)))
    ws, idxs = [], []
    for c in range(N_CORES):
        out = res.results[c]
        ws.append(out["outw"][:VS].reshape(VS, R, A, 3))
        idxs.append(out["outi"][:VS].reshape(VS, R, A, 3))
    w = np.concatenate(ws, 0).astype(np.float32)
    idx = np.rint(np.concatenate(idxs, 0)).astype(np.int32)
    if _want_time:
        return (w, idx), res
    return w, idx



# revision 5
# speedup vs baseline: 2.3396x; 2.3396x over previous
"""Trainium2 Bass kernel for BarycentricCoordinates — v3.

Same fp32 math as v2 (measured decision margins down to 2.4e-7 relative
forbid 16-bit or reordered arithmetic). v3 restructures for engine overlap:
 - front/back software pipeline: front(vt) = DMA + det table + per-r
   section (Pool-heavy + ACT), back(vt-1) = CR/score chain + tail
   (DVE-heavy). Emission interleaves fronts and backs so the in-order
   engine queues always have independent work (kills ping-pong stalls).
 - det e-table split by c-range: c 0:5 chain entirely on DVE, c 5:8
   entirely on Pool — no cross-engine dependencies inside a chunk.
 - ACT does the copies/affines (U48 pack, E8 scatter, w0, penalty, outs).
 - packed ops (same arithmetic, fewer dispatches): pxy/bxy pairs, U1/U2
   fused mults, tdxy, v0xy, ohij, grk.
 - E-table mask folded to one stt: scatter maxE (and -minEA for the
   D-block) into E8(i,j,c), then okb = (E8 <= 0) * NEQP.
"""

import sys

sys.path.insert(0, "/opt/trn_rl_repo")

import numpy as np

import concourse.bass as bass
import concourse.bacc as bacc
import concourse.mybir as mybir
from concourse.tile import TileContext

F32 = mybir.dt.float32
I32 = mybir.dt.int32
I16 = mybir.dt.int16
OP = mybir.AluOpType
AF = mybir.ActivationFunctionType
AX = mybir.AxisListType

BIG = 2.0e38
N_CORES = 8
V_TOTAL = 5000
R, A, K0 = 5, 8, 8
RA = R * A          # 40
VS = V_TOTAL // N_CORES
P = 128
VSP = 640
K2 = 64
RK = RA * K0        # 320
PP = RA * K2        # 2560
CHD = 3             # det c-chunk 0:CHD on DVE; CHD:8 on Pool
Q48 = 48
QW = RA * Q48       # 1920
BLKS = ((0, 4), (0, 0), (4, 4))


def build_nc(vsp=VSP):
    nc = bacc.Bacc("TRN2", target_bir_lowering=False)
    n_vt = vsp // P

    px_d = nc.dram_tensor("px", (vsp, K0), F32, kind="ExternalInput")
    py_d = nc.dram_tensor("py", (vsp, K0), F32, kind="ExternalInput")
    tmpl_d = nc.dram_tensor("tmpl", (2, RA), F32, kind="ExternalInput")
    neqp_d = nc.dram_tensor("neqp", (1, 512), F32, kind="ExternalInput")
    iota8_d = nc.dram_tensor("iota8", (1, K0), F32, kind="ExternalInput")
    woff_d = nc.dram_tensor("woff", (1, RA), mybir.dt.uint32, kind="ExternalInput")
    outw_d = nc.dram_tensor("outw", (vsp, RA, 3), F32, kind="ExternalOutput")
    outi_d = nc.dram_tensor("outi", (vsp, RA, 3), F32, kind="ExternalOutput")

    with TileContext(nc) as tc:
        VE = nc.vector
        GP = nc.gpsimd
        SC = nc.scalar

        def bcv(ap, shape):
            return ap.to_broadcast(shape)

        with (
            tc.tile_pool(name="const", bufs=1) as cpool,
            tc.tile_pool(name="fr3", bufs=3) as f3pool,    # live into back2
            tc.tile_pool(name="fr2", bufs=2) as f2pool,    # live into det
            tc.tile_pool(name="det", bufs=2) as dpool,     # e-table scratch
            tc.tile_pool(name="ij", bufs=1) as ipool,      # back 2560-wide
            tc.tile_pool(name="small", bufs=2) as opool,   # back small tail
        ):
            TXY = cpool.tile([P, 2 * RA], F32, tag="TXY")   # [tx | ty]
            NEQP = cpool.tile([P, 512], F32, tag="NEQP")
            IOTA8 = cpool.tile([P, K0], F32, tag="IOTA8")
            WOFF = cpool.tile([P, RA], mybir.dt.uint32, tag="WOFF")
            nc.sync.dma_start(TXY[:, 0:RA], tmpl_d[0:1, :].to_broadcast((P, RA)))
            nc.sync.dma_start(TXY[:, RA:2 * RA], tmpl_d[1:2, :].to_broadcast((P, RA)))
            nc.sync.dma_start(IOTA8, iota8_d[0:1, :].to_broadcast((P, K0)))

            def front_pool(vt):
                """Per-r section then det U-build. Pool + ACT, with the
                four per-r reduces on DVE (emitted early in DVE's queue)."""
                t = {}
                v0_, v1_ = vt * P, (vt + 1) * P
                pxy = f2pool.tile([P, 2 * K0], F32, tag="pxy")   # [px | py]
                nc.sync.dma_start(pxy[:, 0:K0], px_d[v0_:v1_, :])
                nc.sync.dma_start(pxy[:, K0:2 * K0], py_d[v0_:v1_, :])
                t["pxy"] = pxy
                pxyh = pxy.rearrange("p (h k) -> p h k", h=2)

                # ---- per-r: distances / closest ----
                tdxy = spool.tile([P, 2 * RK], F32, tag="tdxy")
                VE.tensor_tensor(
                    out=tdxy.rearrange("p (h r k) -> p h r k", h=2, k=K0),
                    in0=bcv(pxyh.unsqueeze(2), (P, 2, RA, K0)),
                    in1=bcv(TXY.rearrange("p (h r) -> p h r", h=2).unsqueeze(3),
                            (P, 2, RA, K0)),
                    op=OP.subtract)
                VE.tensor_tensor(out=tdxy, in0=tdxy, in1=tdxy, op=OP.mult)
                d2 = spool.tile([P, RK], F32, tag="d2")
                VE.tensor_tensor(out=d2, in0=tdxy[:, 0:RK],
                                 in1=tdxy[:, RK:2 * RK], op=OP.add)
                d2v = d2.rearrange("p (r k) -> p r k", k=K0)
                dmin = f2pool.tile([P, RA], F32, tag="dmin")
                VE.tensor_reduce(out=dmin, in_=d2v, axis=AX.X, op=OP.min)
                m0 = spool.tile([P, RK], F32, tag="m0")
                m0v = m0.rearrange("p (r k) -> p r k", k=K0)
                VE.tensor_tensor(out=m0v, in0=d2v,
                                 in1=bcv(dmin.unsqueeze(2), (P, RA, K0)),
                                 op=OP.is_equal)

                # gather streams: cx, cy, c_f
                STRM = spool.tile([P, 24], F32, tag="STRM")
                SC.copy(out=STRM[:, 0:16], in_=pxy)
                SC.copy(out=STRM[:, 16:24], in_=IOTA8)
                gm = spool.tile([P, RK * 3], F32, tag="gm")
                gmv = gm.rearrange("p (r g k) -> p r g k", g=3, k=K0)
                VE.tensor_tensor(
                    out=gmv, in0=bcv(m0v.unsqueeze(2), (P, RA, 3, K0)),
                    in1=bcv(STRM.rearrange("p (g k) -> p g k", k=K0).unsqueeze(1),
                            (P, RA, 3, K0)), op=OP.mult)
                g4 = f3pool.tile([P, RA * 3], F32, tag="g4")
                g4v = g4.rearrange("p (r g) -> p r g", g=3)
                VE.tensor_reduce(out=g4v, in_=gmv, axis=AX.X, op=OP.add)
                t["g4"] = g4
                cxy = g4v[:, :, 0:2].rearrange("p r g -> p g r")   # [cx | cy]

                # second closest (fallback index)
                d2b = spool.tile([P, RK], F32, tag="d2b")
                VE.scalar_tensor_tensor(out=d2b, in0=m0, scalar=BIG, in1=d2,
                                        op0=OP.mult, op1=OP.add)
                dmin2 = f2pool.tile([P, RA], F32, tag="dmin2")
                d2bv = d2b.rearrange("p (r k) -> p r k", k=K0)
                VE.tensor_reduce(out=dmin2, in_=d2bv, axis=AX.X, op=OP.min)
                eq2 = spool.tile([P, RK], F32, tag="eq2")
                eq2v = eq2.rearrange("p (r k) -> p r k", k=K0)
                VE.tensor_tensor(out=eq2v, in0=d2bv,
                                 in1=bcv(dmin2.unsqueeze(2), (P, RA, K0)),
                                 op=OP.is_equal)
                VE.tensor_tensor(out=eq2v, in0=eq2v,
                                 in1=bcv(IOTA8.unsqueeze(1), (P, RA, K0)),
                                 op=OP.mult)
                o1_f = f3pool.tile([P, RA], F32, tag="o1_f")
                VE.tensor_reduce(out=o1_f, in_=eq2v, axis=AX.X, op=OP.add)
                t["o1_f"] = o1_f

                # ---- v0 = p - closest (packed), v2 = t - closest ----
                v0xy = f3pool.tile([P, 2 * RK], F32, tag="v0xy")
                VE.tensor_tensor(
                    out=v0xy.rearrange("p (h r k) -> p h r k", h=2, k=K0),
                    in0=bcv(pxyh.unsqueeze(2), (P, 2, RA, K0)),
                    in1=bcv(cxy.unsqueeze(3), (P, 2, RA, K0)),
                    op=OP.subtract)
                t["v0xy"] = v0xy
                v2xy = f3pool.tile([P, 2 * RA], F32, tag="v2xy")
                VE.tensor_tensor(out=v2xy, in0=TXY, in1=cxy, op=OP.subtract)
                t["v2xy"] = v2xy
                # a2(r,k) = v0y*v2x - v0x*v2y
                q1 = spool.tile([P, RK], F32, tag="q1")
                q2 = spool.tile([P, RK], F32, tag="q2")
                VE.tensor_tensor(out=q1.rearrange("p (r k) -> p r k", k=K0),
                                 in0=v0xy[:, RK:2 * RK].rearrange(
                                     "p (r k) -> p r k", k=K0),
                                 in1=bcv(v2xy[:, 0:RA].unsqueeze(2),
                                         (P, RA, K0)), op=OP.mult)
                VE.tensor_tensor(out=q2.rearrange("p (r k) -> p r k", k=K0),
                                 in0=v0xy[:, 0:RK].rearrange(
                                     "p (r k) -> p r k", k=K0),
                                 in1=bcv(v2xy[:, RA:2 * RA].unsqueeze(2),
                                         (P, RA, K0)), op=OP.mult)
                a2 = f3pool.tile([P, RK], F32, tag="a2")
                VE.tensor_tensor(out=a2, in0=q1, in1=q2, op=OP.subtract)
                t["a2"] = a2

                # ---- s = |p|^2 ----
                psq = spool.tile([P, 2 * K0], F32, tag="psq")
                s_ = f2pool.tile([P, K0], F32, tag="s")
                GP.tensor_tensor(out=psq, in0=pxy, in1=pxy, op=OP.mult)
                GP.tensor_tensor(out=s_, in0=psq[:, 0:K0], in1=psq[:, K0:2 * K0],
                                 op=OP.add)

                # ---- b tensors: bxy = [bx | by] at (c,k); bs separate ----
                bxy = f2pool.tile([P, 2 * K2], F32, tag="bxy")
                bsb = f2pool.tile([P, K2], F32, tag="bsb")
                GP.tensor_tensor(
                    out=bxy.rearrange("p (h c k) -> p h c k", h=2, k=K0),
                    in0=bcv(pxyh.unsqueeze(3), (P, 2, K0, K0)),
                    in1=bcv(pxyh.unsqueeze(2), (P, 2, K0, K0)),
                    op=OP.subtract)
                bsv = bsb.rearrange("p (c k) -> p c k", k=K0)
                GP.tensor_tensor(out=bsv, in0=bcv(s_.unsqueeze(2), (P, K0, K0)),
                                 in1=bcv(s_.unsqueeze(1), (P, K0, K0)),
                                 op=OP.subtract)
                bx = bxy[:, 0:K2]
                by = bxy[:, K2:2 * K2]
                t["bx"], t["by"], t["bsb"] = bx, by, bsb

                # ---- U tensors (i,j,k): U21 = [U2 | U1]; U3 separate ----
                # U2 = bx_i*bs_j - bs_i*bx_j ; U1 = by_i*bs_j - bs_i*by_j
                # U3 = bx_i*by_j - by_i*bx_j
                def I_(tt):
                    v = tt.rearrange("p (c k) -> p c k", k=K0)
                    return bcv(v.unsqueeze(2), (P, K0, K0, K0))

                def J_(tt):
                    v = tt.rearrange("p (c k) -> p c k", k=K0)
                    return bcv(v.unsqueeze(1), (P, K0, K0, K0))

                mA = spool.tile([P, 1024], F32, tag="mA")
                mB = spool.tile([P, 1024], F32, tag="mB")
                m5 = spool.tile([P, 512], F32, tag="m5")
                u3t = spool.tile([P, 512], F32, tag="u3t")
                mAv = mA.rearrange("p (h i j k) -> p h i j k", h=2, j=K0, k=K0)
                mBv = mB.rearrange("p (h i j k) -> p h i j k", h=2, j=K0, k=K0)
                for h, bt in ((0, bx), (1, by)):
                    GP.tensor_tensor(out=mAv[:, h], in0=I_(bt), in1=J_(bsb),
                                     op=OP.mult)
                    GP.tensor_tensor(out=mBv[:, h], in0=I_(bsb), in1=J_(bt),
                                     op=OP.mult)
                GP.tensor_tensor(out=mA, in0=mA, in1=mB, op=OP.subtract)
                GP.tensor_tensor(
                    out=m5.rearrange("p (i j k) -> p i j k", j=K0, k=K0),
                    in0=I_(bx), in1=J_(by), op=OP.mult)
                GP.tensor_tensor(
                    out=u3t.rearrange("p (i j k) -> p i j k", j=K0, k=K0),
                    in0=I_(by), in1=J_(bx), op=OP.mult)
                GP.tensor_tensor(out=u3t, in0=m5, in1=u3t, op=OP.subtract)
                U21h = mA.rearrange("p (h i j k) -> p h i j k", h=2, j=K0, k=K0)
                U1v = U21h[:, 1]
                U2v = U21h[:, 0]
                U3v = u3t.rearrange("p (i j k) -> p i j k", j=K0, k=K0)

                # ---- pack U into 48-pair blocks (ACT) ----
                U48s = []
                for un, Utv in enumerate((U1v, U2v, U3v)):
                    t48 = f2pool.tile([P, Q48 * K0], F32, tag=f"U48_{un}")
                    for b, (io, jo) in enumerate(BLKS):
                        SC.copy(out=t48[:, b * 128:(b + 1) * 128].rearrange(
                            "p (a b2 k) -> p a b2 k", b2=4, k=K0),
                                in_=Utv[:, io:io + 4, jo:jo + 4, :])
                    U48s.append(t48)
                t["U48s"] = U48s
                return t

            def det_mults(t, vt):
                """det e-table: c 0:CHD chain on DVE (incl. its reduces),
                CHD:8 mult/sub chain on Pool (reduces deferred). Tile 0
                runs both chunks on DVE so the pipeline fill never blocks
                on Pool's first eB chunk."""
                bx, by, bsb, U48s = t["bx"], t["by"], t["bsb"], t["U48s"]
                maxE = spool.tile([P, Q48 * K0], F32, tag="maxE")
                maxEv = maxE.rearrange("p (q c) -> p q c", c=K0)
                minEA = spool.tile([P, 16 * K0], F32, tag="minEA")
                minEAv = minEA.rearrange("p (q c) -> p q c", c=K0)
                t["maxE"], t["minEA"] = maxE, minEA
                for eng, c0, c1 in ((VE, 0, CHD),
                                    (VE if vt == 0 else GP, CHD, K0)):
                    ch = c1 - c0
                    e1 = dpool.tile([P, Q48 * ch * K0], F32, tag=f"e1_{c0}")
                    e2 = dpool.tile([P, Q48 * ch * K0], F32, tag=f"e2_{c0}")
                    e1v = e1.rearrange("p (q c k) -> p q c k", c=ch, k=K0)
                    e2v = e2.rearrange("p (q c k) -> p q c k", c=ch, k=K0)

                    def Bch(tt, c0=c0, ch=ch):
                        return bcv(tt.rearrange("p (c k) -> p c k", k=K0)
                                   [:, c0:c0 + ch, :].unsqueeze(1),
                                   (P, Q48, ch, K0))

                    def Uch(tt, ch=ch):
                        return bcv(tt.rearrange("p (q k) -> p q k", k=K0)
                                   .unsqueeze(2), (P, Q48, ch, K0))

                    eng.tensor_tensor(out=e1v, in0=Uch(U48s[0]), in1=Bch(bx),
                                      op=OP.mult)
                    eng.tensor_tensor(out=e2v, in0=Uch(U48s[1]), in1=Bch(by),
                                      op=OP.mult)
                    eng.tensor_tensor(out=e1, in0=e1, in1=e2, op=OP.subtract)
                    eng.tensor_tensor(out=e2v, in0=Uch(U48s[2]), in1=Bch(bsb),
                                      op=OP.mult)
                    eng.tensor_tensor(out=e1, in0=e1, in1=e2, op=OP.add)
                    if eng is VE:
                        VE.tensor_reduce(out=maxEv[:, :, c0:c1], in_=e1v,
                                         axis=AX.X, op=OP.max)
                        VE.tensor_reduce(out=minEAv[:, :, c0:c1],
                                         in_=e1v[:, 0:16, :, :], axis=AX.X,
                                         op=OP.min)
                    else:
                        t["e1B"] = (e1, c0, c1)

            def det_finish(t, vt):
                """Deferred Pool-chunk reduces + E8 scatter + okb/okall —
                emitted at the START of the next iteration so nothing
                queues behind a cross-engine wait."""
                maxE, minEA = t["maxE"], t["minEA"]
                maxEv = maxE.rearrange("p (q c) -> p q c", c=K0)
                minEAv = minEA.rearrange("p (q c) -> p q c", c=K0)
                if "e1B" in t:
                    e1, c0, c1 = t["e1B"]
                    ch = c1 - c0
                    e1v = e1.rearrange("p (q c k) -> p q c k", c=ch, k=K0)
                    VE.tensor_reduce(out=maxEv[:, :, c0:c1], in_=e1v,
                                     axis=AX.X, op=OP.max)
                    VE.tensor_reduce(out=minEAv[:, :, c0:c1],
                                     in_=e1v[:, 0:16, :, :], axis=AX.X,
                                     op=OP.min)

                # ---- scatter into E8(i,j,c); D-block negated so one
                # (<=0)*NEQP stt builds the whole validity table ----
                E8 = spool.tile([P, 512], F32, tag="E8")
                E8v = E8.rearrange("p (i j c) -> p i j c", j=K0, c=K0)
                mFv = maxE.rearrange("p (q c) -> p q c", c=K0)
                VE.tensor_copy(out=E8v[:, 0:4, 4:8, :], in_=mFv[:, 0:16, :]
                               .rearrange("p (a b) c -> p a b c", b=4))
                VE.tensor_copy(out=E8v[:, 0:4, 0:4, :], in_=mFv[:, 16:32, :]
                               .rearrange("p (a b) c -> p a b c", b=4))
                VE.tensor_copy(out=E8v[:, 4:8, 4:8, :], in_=mFv[:, 32:48, :]
                               .rearrange("p (a b) c -> p a b c", b=4))
                VE.tensor_scalar(out=E8v[:, 4:8, 0:4, :], in0=minEA.rearrange(
                    "p (a b c) -> p b a c", b=4, c=K0), scalar1=-1.0,
                    scalar2=None, op0=OP.mult)
                okb = spool.tile([P, 512], F32, tag="okb")
                VE.scalar_tensor_tensor(out=okb, in0=E8, scalar=0.0, in1=NEQP,
                                        op0=OP.is_le, op1=OP.mult)
                okall = f3pool.tile([P, K2], F32, tag="okall")
                VE.tensor_reduce(out=okall,
                                 in_=okb.rearrange("p (q c) -> p q c", c=K0),
                                 axis=AX.X, op=OP.add)
                t["okall"] = okall

            def back1(t, vt):
                v0xy, a2t, g4 = t["v0xy"], t["a2"], t["g4"]
                g4v = g4.rearrange("p (r g) -> p r g", g=3)
                c_f = g4v[:, :, 2:3].squeeze(2)
                v0x = v0xy[:, 0:RK]
                v0y = v0xy[:, RK:2 * RK]
                v0xv = v0x.rearrange("p (r k) -> p r k", k=K0)
                v0yv = v0y.rearrange("p (r k) -> p r k", k=K0)
                v2xy = t["v2xy"]
                v2x = v2xy[:, 0:RA]
                v2y = v2xy[:, RA:2 * RA]

                cr0 = ipool.tile([P, PP], F32, tag="cr0")
                cr1 = ipool.tile([P, PP], F32, tag="cr1")
                inv = ipool.tile([P, PP], F32, tag="inv")
                w2 = ipool.tile([P, PP], F32, tag="w2")
                si = ipool.tile([P, PP], I16, tag="si")
                BITF = ipool.tile([P, PP], F32, tag="BITF")

                def blkI(tt, io):
                    return bcv(tt[:, :, io:io + 4].unsqueeze(3), (P, RA, 4, 4))

                def blkJ(tt, jo):
                    return bcv(tt[:, :, jo:jo + 4].unsqueeze(2), (P, RA, 4, 4))

                def q48blk(tt, b):
                    return tt[:, 0:QW].rearrange("p (r q) -> p r q", q=Q48)[
                        :, :, b * 16:(b + 1) * 16].rearrange(
                        "p r (a b2) -> p r a b2", b2=4)

                a2v2 = a2t.rearrange("p (r k) -> p r k", k=K0)
                for b, (io, jo) in enumerate(BLKS):
                    VE.tensor_tensor(out=q48blk(cr0, b), in0=blkI(v0xv, io),
                                     in1=blkJ(v0yv, jo), op=OP.mult)
                for b, (io, jo) in enumerate(BLKS):
                    VE.tensor_tensor(out=q48blk(cr1, b), in0=blkI(v0yv, io),
                                     in1=blkJ(v0xv, jo), op=OP.mult)
                VE.tensor_tensor(out=cr0[:, 0:QW], in0=cr0[:, 0:QW],
                                 in1=cr1[:, 0:QW], op=OP.subtract)   # CR48
                VE.reciprocal_approx_accurate(out=inv[:, 0:QW], in_=cr0[:, 0:QW],
                                              scratch=cr1[:, 0:QW])
                w2v4 = w2.rearrange("p (r i j) -> p r i j", i=K0, j=K0)
                for b, (io, jo) in enumerate(BLKS):
                    VE.tensor_tensor(out=w2v4[:, :, io:io + 4, jo:jo + 4],
                                     in0=blkJ(a2v2, jo), in1=q48blk(inv, b),
                                     op=OP.mult)
                invAT = inv[:, 0:QW].rearrange("p (r q) -> p r q", q=Q48)[
                    :, :, 0:16].rearrange("p r (a b2) -> p r b2 a", b2=4)
                VE.tensor_tensor(out=w2v4[:, :, 4:8, 0:4], in0=blkJ(a2v2, 0),
                                 in1=invAT, op=OP.mult)
                SC.activation(out=w2v4[:, :, 4:8, 0:4],
                              in_=w2v4[:, :, 4:8, 0:4], func=AF.Copy,
                              scale=-1.0)
                # okbit extract after the CR chain: si waits on okall, so
                # emitting it late keeps the DVE queue stall-free.
                okall = t["okall"]
                oki = opool.tile([P, K2], I16, tag="oki")
                SC.copy(out=oki, in_=okall)
                ci = opool.tile([P, RA], I16, tag="ci")
                SC.copy(out=ci, in_=c_f)
                # materialize the ci broadcast (ACT) so the i16 shift's
                # operands are all contiguous -> DVE 2x mode
                cim = spool.tile([P, PP], I16, tag="cim")
                SC.copy(out=cim.rearrange("p (r q) -> p r q", q=K2),
                        in_=bcv(ci.unsqueeze(2), (P, RA, K2)))
                VE.tensor_tensor(out=si.rearrange("p (r q) -> p r q", q=K2),
                                 in0=bcv(oki.unsqueeze(1), (P, RA, K2)),
                                 in1=cim.rearrange("p (r q) -> p r q", q=K2),
                                 op=OP.logical_shift_right)
                VE.tensor_scalar(out=si, in0=si, scalar1=1, scalar2=None,
                                 op0=OP.bitwise_and)

                SC.copy(out=BITF, in_=si)                             # bitf
                t["w2"], t["BITF"] = w2, BITF

            def back2(t, vt):
                v0_, v1_ = vt * P, (vt + 1) * P
                v0xy, g4 = t["v0xy"], t["g4"]
                g4v = g4.rearrange("p (r g) -> p r g", g=3)
                c_f = g4v[:, :, 2:3].squeeze(2)
                v2xy = t["v2xy"]
                w2, BITF = t["w2"], t["BITF"]
                s1a = ipool.tile([P, PP], F32, tag="s1a")
                w0t = ipool.tile([P, PP], F32, tag="w0t")
                mn = ipool.tile([P, PP], F32, tag="mn")

                def T4(tt):
                    return tt.rearrange("p (r i j) -> p r i j", i=K0, j=K0)

                def T4T(tt):
                    return tt.rearrange("p (r j i) -> p r i j", j=K0, i=K0)

                # w1 = transpose(w2); w0 = 1 - w2 - w1
                GP.tensor_tensor(out=T4(s1a), in0=T4(w2), in1=T4T(w2), op=OP.add)
                SC.activation(out=w0t, in_=s1a, func=AF.Copy, bias=1.0, scale=-1.0)
                VE.tensor_tensor(out=T4(mn), in0=T4(w2), in1=T4T(w2), op=OP.min)
                VE.tensor_tensor(out=T4(s1a), in0=T4(w2), in1=T4T(w2), op=OP.max)
                VE.tensor_tensor(out=s1a, in0=s1a, in1=w0t, op=OP.max)  # wmax
                VE.tensor_tensor(out=mn, in0=mn, in1=w0t, op=OP.min)    # wmin
                # valid = (wmin > 0) * bitf ; score = max(wmax, BIG*(1-valid))
                VE.scalar_tensor_tensor(out=mn, in0=mn, scalar=0.0, in1=BITF,
                                        op0=OP.is_gt, op1=OP.mult)
                SC.activation(out=mn, in_=mn, func=AF.Copy, bias=BIG, scale=-BIG)
                VE.tensor_tensor(out=s1a, in0=s1a, in1=mn, op=OP.max)   # score
                smin = opool.tile([P, RA], F32, tag="smin")
                VE.tensor_reduce(out=smin,
                                 in_=s1a.rearrange("p (r q) -> p r q", q=K2),
                                 axis=AX.X, op=OP.min)
                idxw = opool.tile([P, RA], mybir.dt.uint32, tag="idxw")
                for qw in range(RA // 8):
                    VE.max_index(out=idxw[:, 8 * qw:8 * qw + 8],
                                 in_max=smin[:, 8 * qw:8 * qw + 8],
                                 in_values=s1a[:, 512 * qw:512 * (qw + 1)])
                ij_i = opool.tile([P, RA], I32, tag="ij_i")
                VE.tensor_tensor(out=ij_i, in0=idxw, in1=WOFF, op=OP.subtract)

                # ---- tail: fallback, index decode, exact refine ----
                fb = opool.tile([P, RA], F32, tag="fb")
                nfb = opool.tile([P, RA], F32, tag="nfb")
                VE.tensor_scalar(out=fb, in0=smin, scalar1=1.0e38, scalar2=None,
                                 op0=OP.is_ge)
                VE.tensor_scalar(out=nfb, in0=fb, scalar1=-1.0, scalar2=1.0,
                                 op0=OP.mult, op1=OP.add)
                ij2i = opool.tile([P, 2 * RA], I32, tag="ij2i")
                ij2f = opool.tile([P, 2 * RA], F32, tag="ij2f")
                VE.tensor_scalar(out=ij2i[:, 0:RA], in0=ij_i, scalar1=3,
                                 scalar2=None, op0=OP.arith_shift_right)
                VE.tensor_scalar(out=ij2i[:, RA:2 * RA], in0=ij_i, scalar1=7,
                                 scalar2=None, op0=OP.bitwise_and)
                SC.copy(out=ij2f, in_=ij2i)
                i_f = ij2f[:, 0:RA]
                j_f = ij2f[:, RA:2 * RA]

                # one-hots for (i*, j*) and the 4 gather streams, packed
                ohij = spool.tile([P, 2 * RK], F32, tag="ohij")
                VE.tensor_tensor(
                    out=ohij.rearrange("p (h r k) -> p h r k", h=2, k=K0),
                    in0=bcv(IOTA8.unsqueeze(1).unsqueeze(1), (P, 2, RA, K0)),
                    in1=bcv(ij2f.rearrange("p (h r) -> p h r", h=2)
                            .unsqueeze(3), (P, 2, RA, K0)),
                    op=OP.is_equal)
                grk = spool.tile([P, 4 * RK], F32, tag="grk")
                # (h=i/j, g=x/y, r, k) -> after reduce: [xi, yi, xj, yj]
                grkv = grk.rearrange("p (h g r k) -> p h g r k", h=2, g=2, k=K0)
                ohv = ohij.rearrange("p (h r k) -> p h r k", h=2, k=K0)
                v0g = v0xy.rearrange("p (g r k) -> p g r k", g=2, k=K0)
                for h in (0, 1):
                    GP.tensor_tensor(
                        out=grkv[:, h],
                        in0=bcv(ohv[:, h].unsqueeze(1), (P, 2, RA, K0)),
                        in1=v0g, op=OP.mult)
                gx = opool.tile([P, 4 * RA], F32, tag="gx")
                VE.tensor_reduce(out=gx.rearrange("p (g r) -> p g r", g=4),
                                 in_=grk.rearrange("p (gr k) -> p gr k", k=K0),
                                 axis=AX.X, op=OP.add)
                xi = gx[:, 0:RA]
                yi = gx[:, RA:2 * RA]
                xj = gx[:, 2 * RA:3 * RA]
                yj = gx[:, 3 * RA:4 * RA]

                sqg = spool.tile([P, 4 * RA], F32, tag="sqg")
                VE.tensor_tensor(out=sqg, in0=gx, in1=gx, op=OP.mult)
                d00ij = opool.tile([P, 2 * RA], F32, tag="d00ij")
                sq4 = sqg.rearrange("p (a h r) -> p a h r", a=2, h=2)
                VE.tensor_tensor(out=d00ij.rearrange("p (a r) -> p a r", a=2),
                                 in0=sq4[:, :, 0], in1=sq4[:, :, 1], op=OP.add)
                d00i = d00ij[:, 0:RA]
                d00j = d00ij[:, RA:2 * RA]
                prod1 = spool.tile([P, 2 * RA], F32, tag="prod1")
                VE.tensor_tensor(out=prod1, in0=gx[:, 0:2 * RA],
                                 in1=gx[:, 2 * RA:4 * RA], op=OP.mult)
                prod2 = spool.tile([P, 4 * RA], F32, tag="prod2")
                VE.tensor_tensor(
                    out=prod2.rearrange("p (a h r) -> p a h r", a=2, h=2),
                    in0=gx.rearrange("p (a h r) -> p a h r", a=2, h=2),
                    in1=bcv(v2xy.rearrange("p (h r) -> p h r", h=2)
                            .unsqueeze(1), (P, 2, 2, RA)),
                    op=OP.mult)
                dot01r = opool.tile([P, RA], F32, tag="dot01r")
                VE.tensor_tensor(out=dot01r, in0=prod1[:, 0:RA],
                                 in1=prod1[:, RA:2 * RA], op=OP.add)
                d02ij = opool.tile([P, 2 * RA], F32, tag="d02ij")
                pr2 = prod2.rearrange("p (a h r) -> p a h r", a=2, h=2)
                VE.tensor_tensor(out=d02ij.rearrange("p (a r) -> p a r", a=2),
                                 in0=pr2[:, :, 0], in1=pr2[:, :, 1], op=OP.add)
                d02i = d02ij[:, 0:RA]
                d02j = d02ij[:, RA:2 * RA]
                ta = opool.tile([P, RA], F32, tag="ta")
                tb = opool.tile([P, RA], F32, tag="tb")
                denr = opool.tile([P, RA], F32, tag="denr")
                invr = opool.tile([P, RA], F32, tag="invr")
                scr = opool.tile([P, RA], F32, tag="scr")
                VE.tensor_tensor(out=ta, in0=dot01r, in1=dot01r, op=OP.mult)
                VE.tensor_tensor(out=tb, in0=d00i, in1=d00j, op=OP.mult)
                VE.tensor_tensor(out=denr, in0=tb, in1=ta, op=OP.subtract)
                VE.scalar_tensor_tensor(out=denr, in0=denr, scalar=0.0,
                                        in1=denr, op0=OP.is_equal, op1=OP.add)
                VE.reciprocal_approx_accurate(out=invr, in_=denr, scratch=scr)
                P4 = spool.tile([P, 4 * RA], F32, tag="P4")
                P4v = P4.rearrange("p (g r) -> p g r", g=4)
                VE.tensor_tensor(out=P4v[:, 0, :], in0=d00j, in1=d02i, op=OP.mult)
                VE.tensor_tensor(out=P4v[:, 1, :], in0=d00i, in1=d02j, op=OP.mult)
                GP.tensor_tensor(out=P4v[:, 2, :], in0=dot01r, in1=d02j, op=OP.mult)
                GP.tensor_tensor(out=P4v[:, 3, :], in0=dot01r, in1=d02i, op=OP.mult)
                wsel = opool.tile([P, 2 * RA], F32, tag="wsel")
                wselv = wsel.rearrange("p (g r) -> p g r", g=2)
                P4h = P4.rearrange("p (h g r) -> p h g r", h=2, g=2)
                VE.tensor_tensor(out=wselv, in0=P4h[:, 0, :, :],
                                 in1=P4h[:, 1, :, :], op=OP.subtract)
                VE.tensor_tensor(out=wselv, in0=wselv,
                                 in1=bcv(invr.unsqueeze(1), (P, 2, RA)),
                                 op=OP.mult)
                w2sel = wsel[:, 0:RA]
                w1sel = wsel[:, RA:2 * RA]

                o1_f = t["o1_f"]
                VE.copy_predicated(out=i_f, mask=fb.bitcast(I32), data=o1_f)
                VE.copy_predicated(out=j_f, mask=fb.bitcast(I32), data=o1_f)

                w2o = opool.tile([P, RA], F32, tag="w2o")
                w1o = opool.tile([P, RA], F32, tag="w1o")
                w0o = opool.tile([P, RA], F32, tag="w0o")
                VE.tensor_tensor(out=w2o, in0=w2sel, in1=nfb, op=OP.mult)
                VE.tensor_tensor(out=w1o, in0=w1sel, in1=nfb, op=OP.mult)
                VE.tensor_tensor(out=w0o, in0=w2o, in1=w1o, op=OP.add)
                VE.tensor_scalar(out=w0o, in0=w0o, scalar1=-1.0, scalar2=1.0,
                                 op0=OP.mult, op1=OP.add)
                VE.tensor_tensor(out=w0o, in0=w0o, in1=nfb, op=OP.mult)

                wout = opool.tile([P, RA * 3], F32, tag="wout")
                iout = opool.tile([P, RA * 3], F32, tag="iout")
                woutv = wout.rearrange("p (r c) -> p r c", c=3)
                ioutv = iout.rearrange("p (r c) -> p r c", c=3)
                SC.copy(out=woutv[:, :, 0], in_=w0o)
                SC.copy(out=woutv[:, :, 1], in_=w2o)
                SC.copy(out=woutv[:, :, 2], in_=w1o)
                SC.copy(out=ioutv[:, :, 0], in_=c_f)
                SC.copy(out=ioutv[:, :, 1], in_=i_f)
                SC.copy(out=ioutv[:, :, 2], in_=j_f)
                nc.sync.dma_start(outw_d[v0_:v1_, :, :], woutv)
                nc.sync.dma_start(outi_d[v0_:v1_, :, :], ioutv)

            tiles = []
            for vt in range(n_vt):
                if vt >= 1:
                    det_finish(tiles[vt - 1], vt - 1)
                if vt >= 2:
                    back2(tiles[vt - 2], vt - 2)
                tiles.append(front_pool(vt))
                if vt == 0:
                    nc.sync.dma_start(NEQP, neqp_d[0:1, :].to_broadcast((P, 512)))
                    nc.sync.dma_start(WOFF, woff_d[0:1, :].to_broadcast((P, RA)))
                if vt >= 1:
                    back1(tiles[vt - 1], vt - 1)
                det_mults(tiles[vt], vt)
            det_finish(tiles[n_vt - 1], n_vt - 1)
            back2(tiles[n_vt - 2], n_vt - 2)
            back1(tiles[n_vt - 1], n_vt - 1)
            back2(tiles[n_vt - 1], n_vt - 1)

    nc.compile()
    return nc


def make_consts():
    i = np.arange(K0)
    ii = (np.arange(K2) // K0)[:, None]
    jj = (np.arange(K2) % K0)[:, None]
    cc = i[None, :]
    neq = (ii != jj) & (ii != cc) & (jj != cc)          # (64, 8)
    neqp = (neq * (2.0 ** cc)).astype(np.float32).reshape(1, 512)
    iota8 = i.astype(np.float32).reshape(1, K0)
    woff = ((np.arange(RA) % 8) * K2).astype(np.uint32).reshape(1, RA)
    return {"neqp": neqp, "iota8": iota8, "woff": woff}


def make_in_maps(template, projections):
    template = np.ascontiguousarray(np.asarray(template, np.float32))
    projections = np.ascontiguousarray(np.asarray(projections, np.float32))
    consts = make_consts()
    tmplT = np.stack([template[..., 0].reshape(-1), template[..., 1].reshape(-1)])
    px_all = np.ascontiguousarray(projections[..., 0])
    py_all = np.ascontiguousarray(projections[..., 1])
    in_maps = []
    for c in range(N_CORES):
        pxc = px_all[c * VS:(c + 1) * VS]
        pyc = py_all[c * VS:(c + 1) * VS]
        pad = VSP - VS
        pxc = np.concatenate([pxc, np.broadcast_to(pxc[:1], (pad, K0))], 0)
        pyc = np.concatenate([pyc, np.broadcast_to(pyc[:1], (pad, K0))], 0)
        m = {"px": np.ascontiguousarray(pxc), "py": np.ascontiguousarray(pyc),
             "tmpl": tmplT}
        m.update(consts)
        in_maps.append(m)
    return in_maps


_NC_CACHE = {}


def kernel(template, projections, _want_time=False):
    from concourse.bass_utils import run_bass_kernel_spmd
    if "nc" not in _NC_CACHE:
        _NC_CACHE["nc"] = build_nc()
    nc = _NC_CACHE["nc"]
    in_maps = make_in_maps(template, projections)
    res = run_bass_kernel_spmd(nc, in_maps, core_ids=list(range(N_CORES)))
    ws, idxs = [], []
    for c in range(N_CORES):
        out = res.results[c]
        ws.append(out["outw"][:VS].reshape(VS, R, A, 3))
        idxs.append(out["outi"][:VS].reshape(VS, R, A, 3))
    w = np.concatenate(ws, 0).astype(np.float32)
    idx = np.rint(np.concatenate(idxs, 0)).astype(np.int32)
    if _want_time:
        return (w, idx), res
    return w, idx
